# revision 28
# baseline (speedup 1.0000x reference)
"""Distributed Trainium2 kernel for a causal multi-head self-attention block.

  out = softmax_causal((x@Wq+bq)(x@Wk+bk)^T / sqrt(Dh)) (x@Wv+bv) @ W_out + b_out

Sharding (8 NeuronCores, tensor-parallel over heads):
  - Each core owns 2 of the 16 heads, both batches -> 4 (batch, head) units.
  - QKV projection computed in transposed layout (features on partitions)
    from a host-pretransposed xH. Chunks are processed in PAIRS sharing one
    LDWEIGHTS per (head, d) step (stationary reuse), accumulating into the
    two banks of a [128,1024] PSUM tile.
  - qT/kT/vv are aliased across batches (batch 1 reuses batch 0's slices
    after attention(0,1) has consumed them) to fit SBUF.
  - Attention: scoresT = kT-tile.T @ qT (t on partitions, s free). Score
    tiles for tt-pairs land in the two banks of one PSUM tile so a single
    ACTIVATE Exp covers both (the scalar engine's ~352-cycle per-instruction
    overhead was the attention-phase bottleneck). Causal at tile granularity
    with a triangular mask multiply on diagonal tiles; PV matmul uses
    [v | ones] so the softmax denominator falls out of PSUM column 128.
  - Attention output is normalized, PE-transposed to [Dh, s], v-bias added,
    then a per-(batch, head) AllToAll redistributes head-shards ->
    token-shards. The final collective's flight is covered by proj(0) plus
    the h0-half of proj(1) (split accumulation with bf16 partials).
  - Output projection is token-parallel with the full W_out; host
    reassembles, transposes, and adds b_out.

All DRAM-side layouts are partition-major so every DMA is contiguous.
All matmul operands are bf16 (1 cycle/row on the PE), accumulation f32.
"""

import math
import numpy as np
import ml_dtypes

import sys

for _p in ("/opt/trn_rl_repo",):
    if _p not in sys.path:
        sys.path.insert(0, _p)

import concourse.bass as bass
import concourse.bacc as bacc
import concourse.mybir as mybir
import concourse.tile as tile
from concourse.bass_utils import run_bass_kernel_spmd

BF16 = mybir.dt.bfloat16
F32 = mybir.dt.float32
NPBF16 = ml_dtypes.bfloat16

B, S, D = 2, 2048, 2048
H, DH = 16, 128
NC = 8
HL = H // NC            # heads per core = 2
SC = 512                # s-chunk (free dim of scores matmul)
NT = S // 128           # 16 t-tiles per batch
TOKB = S // NC          # 256 tokens owned per core per batch
INV_SQRT_DH = 1.0 / math.sqrt(DH)


def build_kernel(debug=False):
    nc = bacc.Bacc("TRN2", target_bir_lowering=False, debug=False, num_devices=NC)

    xH = nc.declare_dram_parameter("xH", [B, 128, 16, S], BF16, isOutput=False)
    wq = nc.declare_dram_parameter("wq", [128, HL, 16, 128], BF16, isOutput=False)
    wk = nc.declare_dram_parameter("wk", [128, HL, 16, 128], BF16, isOutput=False)
    wv = nc.declare_dram_parameter("wv", [128, 16, HL * 128], BF16, isOutput=False)
    bq = nc.declare_dram_parameter("bq", [128, HL, 1], F32, isOutput=False)
    bk = nc.declare_dram_parameter("bk", [128, HL, 1], F32, isOutput=False)
    bv = nc.declare_dram_parameter("bv", [128, HL, 1], F32, isOutput=False)
    w_out = nc.declare_dram_parameter("w_out", [128, 16, D], BF16, isOutput=False)
    ident = nc.declare_dram_parameter("ident", [128, 128], BF16, isOutput=False)
    maskp = nc.declare_dram_parameter("maskp", [128, 128], BF16, isOutput=False)
    out = nc.declare_dram_parameter("out", [B, D, TOKB], BF16, isOutput=True)

    with tile.TileContext(nc) as tc:
        with (
            tc.tile_pool(name="wpool", bufs=1) as wpool,
            tc.tile_pool(name="xpool", bufs=3) as xpool,
            tc.tile_pool(name="qkv", bufs=1) as qkvpool,
            tc.tile_pool(name="expp", bufs=4) as expp,
            tc.tile_pool(name="small", bufs=4) as small,
            tc.tile_pool(name="wo", bufs=1) as wopool,
            tc.tile_pool(name="rcv", bufs=1) as rcvpool,
            tc.tile_pool(name="outp", bufs=4) as outp,
            tc.tile_pool(name="psum", bufs=2, space="PSUM") as psum,
            tc.tile_pool(name="dram", bufs=1, space="DRAM") as dram,
        ):
            # ---- weights first (all contiguous loads), big ones leading ----
            wq_t = wpool.tile([128, HL, 16, 128], BF16, tag="wq")
            wk_t = wpool.tile([128, HL, 16, 128], BF16, tag="wk")
            wv_t = wpool.tile([128, 16, HL * 128], BF16, tag="wv")
            bq_t = wpool.tile([128, HL, 1], F32, tag="bq")
            bk_t = wpool.tile([128, HL, 1], F32, tag="bk")
            bv_t = wpool.tile([128, HL, 1], F32, tag="bv")
            id_t = wpool.tile([128, 128], BF16, tag="ident")
            mask_t = wpool.tile([128, 128], BF16, tag="maskp")
            for hl in range(HL):
                for dh_ in range(2):
                    nc.gpsimd.dma_start(
                        wq_t[:, hl, 8 * dh_ : 8 * dh_ + 8],
                        wq[:, hl, 8 * dh_ : 8 * dh_ + 8],
                    )
            nc.gpsimd.dma_start(wk_t[:], wk[:])
            nc.gpsimd.dma_start(wv_t[:], wv[:])
            nc.gpsimd.dma_start(bq_t[:], bq[:])
            nc.gpsimd.dma_start(bk_t[:], bk[:])
            nc.gpsimd.dma_start(bv_t[:], bv[:])
            nc.gpsimd.dma_start(id_t[:], ident[:])
            nc.gpsimd.dma_start(mask_t[:], maskp[:])

            # ---- persistent activations (aliased across batches) ----
            qT = qkvpool.tile([128, HL, S], BF16, tag="qT")
            kT = qkvpool.tile([128, HL, S], BF16, tag="kT")
            vv = qkvpool.tile([128, HL, NT, 129], BF16, tag="vv")
            nc.gpsimd.memset(vv[:, :, :, 128:129], 1.0)

            # per-(batch, head) A2A bounce buffers (DRAM): shard per dst core
            # is [128(dh), 256(two s-tiles)] -> fully contiguous stores/loads.
            a2a_in = [
                dram.tile([NC, 128, 2 * 128], BF16, tag=f"a2a_in{u}",
                          name=f"a2a_in{u}")
                for u in range(B * HL)
            ]
            a2a_out = [
                dram.tile([NC, 128, 2 * 128], BF16, tag=f"a2a_out{u}",
                          name=f"a2a_out{u}")
                for u in range(B * HL)
            ]

            def load_chunk(b, tcn, fine=False):
                xt = xpool.tile([128, 16, SC], BF16, tag="xt",
                                name=f"xt_{b}_{tcn}")
                gd = 2 if fine else 4  # d-groups per DMA
                for g in range(16 // gd):
                    nc.sync.dma_start(
                        xt[:, gd * g : gd * g + gd],
                        xH[b, :, gd * g : gd * g + gd,
                           tcn * SC : (tcn + 1) * SC],
                    )
                return xt

            def qkv_v_chunk(b, tcn, xt):
                ps = psum.tile([128, 2 * SC], F32, tag="mm",
                               name=f"psv_{b}_{tcn}")
                for ts in range(SC // 128):
                    for d in range(16):
                        nc.tensor.matmul(
                            ps[:, ts * 256 : ts * 256 + HL * 128],
                            xt[:, d, ts * 128 : (ts + 1) * 128],
                            wv_t[:, d],
                            start=(d == 0 and ts % 2 == 0),
                            stop=(d == 15),
                        )
                for ts in range(SC // 128):
                    tt_idx = tcn * (SC // 128) + ts
                    for hl in range(HL):
                        nc.vector.tensor_copy(
                            vv[:, hl, tt_idx, 0:128],
                            ps[:, ts * 256 + hl * 128 : ts * 256 + (hl + 1) * 128],
                        )

            def qkv_single(b, tcn, xt):
                """Full Q/K/V for one chunk (startup path: compute on the
                resident chunk covers the next chunk's DMA arrival)."""
                for w_t, b_t, dst in ((wq_t, bq_t, qT), (wk_t, bk_t, kT)):
                    for hl in range(HL):
                        ps = psum.tile([128, 2 * SC], F32, tag="mm",
                                       name=f"psqk1_{b}_{tcn}_{hl}_{id(dst)}")
                        for d in range(16):
                            nc.tensor.matmul(
                                ps[:, :SC], w_t[:, hl, d], xt[:, d],
                                start=(d == 0), stop=(d == 15),
                            )
                        nc.vector.tensor_scalar_add(
                            dst[:, hl, tcn * SC : (tcn + 1) * SC],
                            ps[:, :SC], b_t[:, hl],
                        )
                qkv_v_chunk(b, tcn, xt)

            def qkv_pair(b, tcn0, xts, lead=False):
                """Q/K/V for chunks (tcn0, tcn0+1), interleaved so each
                LDWEIGHTS serves two MMs. lead=True keeps Q chunk-major so
                compute on the resident chunk covers the second chunk's DMA."""
                chunks = (0, 1)
                for w_t, b_t, dst in ((wq_t, bq_t, qT), (wk_t, bk_t, kT)):
                    for hl in range(HL):
                        ps = psum.tile([128, 2 * SC], F32, tag="mm",
                                       name=f"psqk_{b}_{tcn0}_{hl}_{id(dst)}")
                        if lead and dst is qT:
                            for ci in chunks:
                                for d in range(16):
                                    nc.tensor.matmul(
                                        ps[:, ci * SC : (ci + 1) * SC],
                                        w_t[:, hl, d], xts[ci][:, d],
                                        start=(d == 0), stop=(d == 15),
                                    )
                        else:
                            for d in range(16):
                                for ci in chunks:
                                    nc.tensor.matmul(
                                        ps[:, ci * SC : (ci + 1) * SC],
                                        w_t[:, hl, d], xts[ci][:, d],
                                        start=(d == 0), stop=(d == 15),
                                    )
                        for ci in chunks:
                            tcn = tcn0 + ci
                            nc.vector.tensor_scalar_add(
                                dst[:, hl, tcn * SC : (tcn + 1) * SC],
                                ps[:, ci * SC : (ci + 1) * SC], b_t[:, hl],
                            )
                for ci in chunks:
                    qkv_v_chunk(b, tcn0 + ci, xts[ci])

            def qkv_phase(b, pair_first):
                for p in range(2):
                    tcn0 = 2 * p
                    xts = (load_chunk(b, tcn0,
                                      fine=(p == 0 and not pair_first)),
                           load_chunk(b, tcn0 + 1))
                    if p == 0 and not pair_first:
                        # chunk-major: compute on chunk 0 covers chunk 1's DMA
                        qkv_single(b, tcn0, xts[0])
                        qkv_single(b, tcn0 + 1, xts[1])
                    else:
                        qkv_pair(b, tcn0, xts,
                                 lead=(not pair_first and p == 1))

            def attn_normalize(u, scn, o2, hl):
                for pj in range(2):  # s-tile pairs; one contiguous store
                    at2 = small.tile([128, 2 * 128], BF16, tag="at2",
                                     name=f"at2_{u}_{scn}_{pj}")
                    for s2 in range(2):
                        ss = 2 * pj + s2
                        o2t = o2[pj]
                        rc = small.tile([128, 1], F32, tag="rc",
                                        name=f"rc_{u}_{scn}_{ss}")
                        nc.vector.reciprocal(rc[:], o2t[:, s2, 128:129])
                        an = small.tile([128, 128], BF16, tag="an",
                                        name=f"an_{u}_{scn}_{ss}")
                        nc.vector.tensor_scalar_mul(
                            an[:], o2t[:, s2, 0:128], rc[:]
                        )
                        tp = psum.tile([128, 128], BF16, tag="tp", bufs=2,
                                       name=f"tp_{u}_{scn}_{ss}")
                        nc.tensor.transpose(tp[:], an[:], id_t[:])
                        nc.vector.tensor_scalar_add(
                            at2[:, s2 * 128 : (s2 + 1) * 128],
                            tp[:], bv_t[:, hl],
                        )
                    # gpsimd is safe: the collective doorbell is a brief
                    # WRITE, it does not block the gpsimd queue
                    nc.gpsimd.dma_start(a2a_in[u][2 * scn + pj], at2[:])

            def attention_unit(u):
                b, hl = u // HL, u % HL
                pending = None  # (scn, o2) awaiting normalize; emitted after
                # the NEXT scn's first scores+exp so the scalar engine has
                # work while the PE runs the normalize transposes.
                for scn in range(S // SC):
                    o2 = None
                    npair = 2 * scn + 2
                    for pr in range(npair):
                        sp = psum.tile([128, 2 * SC], F32, tag="mm",
                                       name=f"sp_{u}_{scn}_{pr}")
                        lives = []
                        for half in range(2):
                            tt = 2 * pr + half
                            off = max(0, tt - 4 * scn)
                            nlive = 4 - off
                            lives.append((tt, off, nlive))
                            s0 = scn * SC + off * 128
                            nc.tensor.matmul(
                                sp[:, half * SC : half * SC + nlive * 128],
                                kT[:, hl, tt * 128 : (tt + 1) * 128],
                                qT[:, hl, s0 : (scn + 1) * SC],
                                start=True, stop=True,
                            )
                        ex = expp.tile([128, 2 * SC], BF16, tag="ex",
                                       name=f"ex_{u}_{scn}_{pr}")
                        ncols = SC + lives[1][2] * 128  # through 2nd half live
                        nc.scalar.activation(
                            ex[:, :ncols], sp[:, :ncols],
                            mybir.ActivationFunctionType.Exp,
                            scale=INV_SQRT_DH,
                        )
                        for half in range(2):
                            tt, off, nlive = lives[half]
                            base = half * SC
                            if tt >= 4 * scn:  # diagonal: causal mask
                                nc.vector.tensor_mul(
                                    ex[:, base : base + 128],
                                    ex[:, base : base + 128], mask_t[:],
                                )
                        if pr == 0:
                            if pending is not None:
                                attn_normalize(u, pending[0], pending[1], hl)
                            o2 = [
                                psum.tile([128, 2, 129], F32, tag="o2",
                                          bufs=2, name=f"o2_{u}_{scn}_{i}")
                                for i in range(2)
                            ]
                        for half in range(2):
                            tt, off, nlive = lives[half]
                            base = half * SC
                            for ss in range(off, 4):
                                st = 4 * scn + ss
                                # start=True clears has_written BANK-wide;
                                # only the first matmul touching each o2 bank
                                # may set it (sibling overwrites rely on the
                                # cleared has_written bits).
                                nc.tensor.matmul(
                                    o2[ss // 2][:, ss % 2, :],
                                    ex[:, base + (ss - off) * 128
                                       : base + (ss - off + 1) * 128],
                                    vv[:, hl, tt],
                                    start=(tt == 0 and ss % 2 == 0),
                                    stop=(tt == st),
                                )
                    pending = (scn, o2)
                attn_normalize(u, pending[0], pending[1], hl)

            def a2a_fire(u):
                nc.gpsimd.collective_compute(
                    "AllToAll",
                    mybir.AluOpType.bypass,
                    ins=[a2a_in[u].opt()],
                    outs=[a2a_out[u].opt()],
                    replica_groups=[list(range(NC))],
                )

            def load_rcv(b, hl, rcv):
                # gpsimd: idle after its collective doorbells, and keeping
                # these off sync/scalar stops them blocking the out-stores
                for srcc in range(NC):
                    nc.gpsimd.dma_start(
                        rcv[:, srcc * HL + hl], a2a_out[b * HL + hl][srcc]
                    )

            def store_out(b, oc, ot):
                eng = nc.sync if oc % 2 == 0 else nc.scalar
                eng.dma_start(out[b, oc * 128 : (oc + 1) * 128, :], ot[:])

            def proj_phase(b, rcv):
                for oc in range(16):
                    ps = psum.tile([128, 2 * SC], F32, tag="mm",
                                   name=f"pso_{b}_{oc}")
                    for dt in range(16):
                        nc.tensor.matmul(
                            ps[:, :TOKB],
                            wo_t[:, dt, oc * 128 : (oc + 1) * 128],
                            rcv[:, dt],
                            start=(dt == 0), stop=(dt == 15),
                        )
                    ot = outp.tile([128, TOKB], BF16, tag="ot",
                                   name=f"ot_{b}_{oc}")
                    nc.vector.tensor_copy(ot[:], ps[:, :TOKB])
                    store_out(b, oc, ot)

            def proj_half_a(b, rcv, pp):
                # h0 (even dt) halves of every oc group -> bf16 partials;
                # runs while the last collective is in flight.
                for oc in range(16):
                    ps = psum.tile([128, 2 * SC], F32, tag="mm",
                                   name=f"psa_{b}_{oc}")
                    for i, dt in enumerate(range(0, 16, 2)):
                        nc.tensor.matmul(
                            ps[:, :TOKB],
                            wo_t[:, dt, oc * 128 : (oc + 1) * 128],
                            rcv[:, dt],
                            start=(i == 0), stop=(i == 7),
                        )
                    nc.vector.tensor_copy(pp[:, oc], ps[:, :TOKB])

            def proj_half_b(b, rcv, pp):
                for oc in range(16):
                    ps = psum.tile([128, 2 * SC], F32, tag="mm",
                                   name=f"psb_{b}_{oc}")
                    for i, dt in enumerate(range(1, 16, 2)):
                        nc.tensor.matmul(
                            ps[:, :TOKB],
                            wo_t[:, dt, oc * 128 : (oc + 1) * 128],
                            rcv[:, dt],
                            start=(i == 0), stop=(i == 7),
                        )
                    ot = outp.tile([128, TOKB], BF16, tag="ot",
                                   name=f"otb_{b}_{oc}")
                    nc.vector.tensor_add(ot[:], ps[:, :TOKB], pp[:, oc])
                    store_out(b, oc, ot)

            # ---------------- program order ----------------
            qkv_phase(0, pair_first=False)
            attention_unit(0)
            a2a_fire(0)
            attention_unit(1)
            a2a_fire(1)
            # W_out load: big (8.4 MB). The scheduler hoists dependency-free
            # DMAs to t=0 where this starved the startup-critical loads, so
            # give it a WAW dependency on late-qkv(0) data via a dummy write.
            wo_t = wopool.tile([128, 16, D], BF16, tag="wo")
            nc.vector.tensor_copy(wo_t[:, 0, 0:1], qT[:, 0, S - 1 : S])
            nc.scalar.dma_start(wo_t[:], w_out[:])
            # rcv0 loads fire as soon as each b0 per-head collective lands
            # (during b1 QKV/attention), so proj(b0) fills the tail bubble.
            rcv0 = rcvpool.tile([128, 16, TOKB], BF16, tag="rcv0")
            load_rcv(0, 0, rcv0)
            load_rcv(0, 1, rcv0)
            qkv_phase(1, pair_first=True)
            attention_unit(2)
            a2a_fire(2)
            attention_unit(3)
            a2a_fire(3)
            # emitted after the last doorbell: gpsimd is idle from here, so
            # these never queue ahead of attention's at2 stores
            rcv1 = rcvpool.tile([128, 16, TOKB], BF16, tag="rcv1")
            load_rcv(1, 0, rcv1)
            proj_phase(0, rcv0)
            load_rcv(1, 1, rcv1)
            pp1 = rcvpool.tile([128, 16, TOKB], BF16, tag="pp1")
            proj_half_a(1, rcv1, pp1)
            proj_half_b(1, rcv1, pp1)

    nc.compile()
    return nc


def make_in_maps(x, W_in, b_in, W_out, b_out):
    # xH[b, p, d, s] = x[b, s, d*128+p]  -> contiguous [128, 16, SC] chunk DMAs
    xH = np.ascontiguousarray(
        x.reshape(B, S, 16, 128).transpose(0, 3, 2, 1)
    ).astype(NPBF16)
    ident = np.eye(128, dtype=NPBF16)
    maskp = np.triu(np.ones((128, 128), dtype=np.float32)).astype(NPBF16)
    # w_out[p, dt, oc] = W_out[dt*128+p, oc] -> contiguous per-partition 64KB
    w_out_t = np.ascontiguousarray(
        W_out.reshape(16, 128, D).transpose(1, 0, 2)
    ).astype(NPBF16)

    in_maps = []
    for c in range(NC):
        hs = [2 * c + hl for hl in range(HL)]  # global head ids
        # wq[p, hl, d, m] = W_in[d*128+p, h*128+m]
        wq_c = np.ascontiguousarray(
            np.stack(
                [W_in[:, h * 128 : (h + 1) * 128].reshape(16, 128, 128) for h in hs]
            ).transpose(2, 0, 1, 3)
        ).astype(NPBF16)
        wk_c = np.ascontiguousarray(
            np.stack(
                [W_in[:, D + h * 128 : D + (h + 1) * 128].reshape(16, 128, 128)
                 for h in hs]
            ).transpose(2, 0, 1, 3)
        ).astype(NPBF16)
        wv_c = np.ascontiguousarray(
            np.concatenate(
                [
                    W_in[:, 2 * D + h * 128 : 2 * D + (h + 1) * 128].reshape(
                        16, 128, 128
                    )
                    for h in hs
                ],
                axis=2,
            ).transpose(1, 0, 2)
        ).astype(NPBF16)
        bq_c = np.stack([b_in[h * 128 : (h + 1) * 128] for h in hs], axis=1).reshape(
            128, HL, 1
        ).astype(np.float32)
        bk_c = np.stack(
            [b_in[D + h * 128 : D + (h + 1) * 128] for h in hs], axis=1
        ).reshape(128, HL, 1).astype(np.float32)
        bv_c = np.stack(
            [b_in[2 * D + h * 128 : 2 * D + (h + 1) * 128] for h in hs], axis=1
        ).reshape(128, HL, 1).astype(np.float32)
        in_maps.append(
            {
                "xH": xH,
                "wq": wq_c,
                "wk": wk_c,
                "wv": wv_c,
                "bq": bq_c,
                "bk": bk_c,
                "bv": bv_c,
                "w_out": w_out_t,
                "ident": ident,
                "maskp": maskp,
            }
        )
    return in_maps


_NC_CACHE = {}


def _get_nc(debug=False):
    key = f"nc{debug}"
    if key not in _NC_CACHE:
        _NC_CACHE[key] = build_kernel(debug)
    return _NC_CACHE[key]


def kernel(x, W_in, b_in, W_out, b_out, _trace=False, _debug=False, **kw):
    x = np.asarray(x, dtype=np.float32)
    W_in = np.asarray(W_in, dtype=np.float32)
    b_in = np.asarray(b_in, dtype=np.float32)
    W_out = np.asarray(W_out, dtype=np.float32)
    b_out = np.asarray(b_out, dtype=np.float32)

    nc = _get_nc(_debug)
    in_maps = make_in_maps(x, W_in, b_in, W_out, b_out)
    res = run_bass_kernel_spmd(nc, in_maps, core_ids=list(range(NC)), trace=_trace)
    outf = np.empty((B, S, D), dtype=np.float32)
    for c in range(NC):
        o = np.asarray(res.results[c]["out"]).astype(np.float32)  # [B, D, TOKB]
        for b in range(B):
            outf[b, c * TOKB : (c + 1) * TOKB, :] = o[b].T
    outf += b_out[None, None, :]
    if _trace or _debug:
        return outf, res
    return outf


# revision 31
# speedup vs baseline: 1.0177x; 1.0177x over previous
"""Distributed Trainium2 kernel for a causal multi-head self-attention block.

  out = softmax_causal((x@Wq+bq)(x@Wk+bk)^T / sqrt(Dh)) (x@Wv+bv) @ W_out + b_out

Sharding (8 NeuronCores, tensor-parallel over heads):
  - Each core owns 2 of the 16 heads, both batches -> 4 (batch, head) units.
  - QKV projection computed in transposed layout (features on partitions)
    from a host-pretransposed xH. Chunks are processed in PAIRS sharing one
    LDWEIGHTS per (head, d) step (stationary reuse), accumulating into the
    two banks of a [128,1024] PSUM tile.
  - qT/kT/vv are aliased across batches (batch 1 reuses batch 0's slices
    after attention(0,1) has consumed them) to fit SBUF.
  - Attention: scoresT = kT-tile.T @ qT (t on partitions, s free). Score
    tiles for tt-pairs land in the two banks of one PSUM tile so a single
    ACTIVATE Exp covers both (the scalar engine's ~352-cycle per-instruction
    overhead was the attention-phase bottleneck). Causal at tile granularity
    with a triangular mask multiply on diagonal tiles; PV matmul uses
    [v | ones] so the softmax denominator falls out of PSUM column 128.
  - Attention output is normalized, PE-transposed to [Dh, s], v-bias added,
    then a per-(batch, head) AllToAll redistributes head-shards ->
    token-shards. The final collective's flight is covered by proj(0) plus
    the h0-half of proj(1) (split accumulation with bf16 partials).
  - Output projection is token-parallel with the full W_out; host
    reassembles, transposes, and adds b_out.

All DRAM-side layouts are partition-major so every DMA is contiguous.
All matmul operands are bf16 (1 cycle/row on the PE), accumulation f32.
"""

import math
import numpy as np
import ml_dtypes

import sys

for _p in ("/opt/trn_rl_repo",):
    if _p not in sys.path:
        sys.path.insert(0, _p)

import concourse.bass as bass
import concourse.bacc as bacc
import concourse.mybir as mybir
import concourse.tile as tile
from concourse.bass_utils import run_bass_kernel_spmd

BF16 = mybir.dt.bfloat16
F32 = mybir.dt.float32
NPBF16 = ml_dtypes.bfloat16

B, S, D = 2, 2048, 2048
H, DH = 16, 128
NC = 8
HL = H // NC            # heads per core = 2
SC = 512                # s-chunk (free dim of scores matmul)
NT = S // 128           # 16 t-tiles per batch
TOKB = S // NC          # 256 tokens owned per core per batch
INV_SQRT_DH = 1.0 / math.sqrt(DH)


def build_kernel(debug=False):
    nc = bacc.Bacc("TRN2", target_bir_lowering=False, debug=False, num_devices=NC)

    xH = nc.declare_dram_parameter("xH", [B, 128, 16, S], BF16, isOutput=False)
    wq = nc.declare_dram_parameter("wq", [128, HL, 16, 128], BF16, isOutput=False)
    wk = nc.declare_dram_parameter("wk", [128, HL, 16, 128], BF16, isOutput=False)
    wv = nc.declare_dram_parameter("wv", [128, 16, HL * 128], BF16, isOutput=False)
    bq = nc.declare_dram_parameter("bq", [128, HL, 1], F32, isOutput=False)
    bk = nc.declare_dram_parameter("bk", [128, HL, 1], F32, isOutput=False)
    bv = nc.declare_dram_parameter("bv", [128, HL, 1], F32, isOutput=False)
    w_out = nc.declare_dram_parameter("w_out", [128, 16, D], BF16, isOutput=False)
    ident = nc.declare_dram_parameter("ident", [128, 128], BF16, isOutput=False)
    maskp = nc.declare_dram_parameter("maskp", [128, 128], BF16, isOutput=False)
    out = nc.declare_dram_parameter("out", [B, D, TOKB], BF16, isOutput=True)

    with tile.TileContext(nc) as tc:
        with (
            tc.tile_pool(name="wpool", bufs=1) as wpool,
            tc.tile_pool(name="xpool", bufs=3) as xpool,
            tc.tile_pool(name="qkv", bufs=1) as qkvpool,
            tc.tile_pool(name="expp", bufs=4) as expp,
            tc.tile_pool(name="small", bufs=4) as small,
            tc.tile_pool(name="wo", bufs=1) as wopool,
            tc.tile_pool(name="rcv", bufs=1) as rcvpool,
            tc.tile_pool(name="outp", bufs=4) as outp,
            tc.tile_pool(name="psum", bufs=2, space="PSUM") as psum,
            tc.tile_pool(name="dram", bufs=1, space="DRAM") as dram,
        ):
            # ---- weights first (all contiguous loads), big ones leading ----
            wq_t = wpool.tile([128, HL, 16, 128], BF16, tag="wq")
            wk_t = wpool.tile([128, HL, 16, 128], BF16, tag="wk")
            wv_t = wpool.tile([128, 16, HL * 128], BF16, tag="wv")
            bq_t = wpool.tile([128, HL, 1], F32, tag="bq")
            bk_t = wpool.tile([128, HL, 1], F32, tag="bk")
            bv_t = wpool.tile([128, HL, 1], F32, tag="bv")
            id_t = wpool.tile([128, 128], BF16, tag="ident")
            mask_t = wpool.tile([128, 128], BF16, tag="maskp")
            nc.gpsimd.dma_start(wq_t[:, 0], wq[:, 0])
            nc.gpsimd.dma_start(wq_t[:, 1], wq[:, 1])
            nc.gpsimd.dma_start(wk_t[:], wk[:])
            nc.gpsimd.dma_start(wv_t[:], wv[:])
            nc.gpsimd.dma_start(bq_t[:], bq[:])
            nc.gpsimd.dma_start(bk_t[:], bk[:])
            nc.gpsimd.dma_start(bv_t[:], bv[:])
            nc.gpsimd.dma_start(id_t[:], ident[:])
            nc.gpsimd.dma_start(mask_t[:], maskp[:])

            # ---- persistent activations (aliased across batches) ----
            qT = qkvpool.tile([128, HL, S], BF16, tag="qT")
            kT = qkvpool.tile([128, HL, S], BF16, tag="kT")
            vv = qkvpool.tile([128, HL, NT, 129], BF16, tag="vv")
            nc.gpsimd.memset(vv[:, :, :, 128:129], 1.0)

            # per-(batch, head) A2A bounce buffers (DRAM): shard per dst core
            # is [128(dh), 256(two s-tiles)] -> fully contiguous stores/loads.
            a2a_in = [
                dram.tile([NC, 128, 2 * 128], BF16, tag=f"a2a_in{u}",
                          name=f"a2a_in{u}")
                for u in range(B * HL)
            ]
            a2a_out = [
                dram.tile([NC, 128, 2 * 128], BF16, tag=f"a2a_out{u}",
                          name=f"a2a_out{u}")
                for u in range(B * HL)
            ]

            def load_chunk(b, tcn):
                xt = xpool.tile([128, 16, SC], BF16, tag="xt",
                                name=f"xt_{b}_{tcn}")
                for g in range(4):
                    nc.sync.dma_start(
                        xt[:, 4 * g : 4 * g + 4],
                        xH[b, :, 4 * g : 4 * g + 4,
                           tcn * SC : (tcn + 1) * SC],
                    )
                return xt

            def qkv_v_chunk(b, tcn, xt):
                ps = psum.tile([128, 2 * SC], F32, tag="mm",
                               name=f"psv_{b}_{tcn}")
                for ts in range(SC // 128):
                    for d in range(16):
                        nc.tensor.matmul(
                            ps[:, ts * 256 : ts * 256 + HL * 128],
                            xt[:, d, ts * 128 : (ts + 1) * 128],
                            wv_t[:, d],
                            start=(d == 0 and ts % 2 == 0),
                            stop=(d == 15),
                        )
                for ts in range(SC // 128):
                    tt_idx = tcn * (SC // 128) + ts
                    for hl in range(HL):
                        nc.vector.tensor_copy(
                            vv[:, hl, tt_idx, 0:128],
                            ps[:, ts * 256 + hl * 128 : ts * 256 + (hl + 1) * 128],
                        )

            def qkv_single(b, tcn, xt):
                """Full Q/K/V for one chunk (startup path: compute on the
                resident chunk covers the next chunk's DMA arrival)."""
                for w_t, b_t, dst in ((wq_t, bq_t, qT), (wk_t, bk_t, kT)):
                    for hl in range(HL):
                        ps = psum.tile([128, 2 * SC], F32, tag="mm",
                                       name=f"psqk1_{b}_{tcn}_{hl}_{id(dst)}")
                        for d in range(16):
                            nc.tensor.matmul(
                                ps[:, :SC], w_t[:, hl, d], xt[:, d],
                                start=(d == 0), stop=(d == 15),
                            )
                        nc.vector.tensor_scalar_add(
                            dst[:, hl, tcn * SC : (tcn + 1) * SC],
                            ps[:, :SC], b_t[:, hl],
                        )
                qkv_v_chunk(b, tcn, xt)

            def qkv_pair(b, tcn0, xts, lead=False):
                """Q/K/V for chunks (tcn0, tcn0+1), interleaved so each
                LDWEIGHTS serves two MMs. lead=True keeps Q chunk-major so
                compute on the resident chunk covers the second chunk's DMA."""
                chunks = (0, 1)
                for w_t, b_t, dst in ((wq_t, bq_t, qT), (wk_t, bk_t, kT)):
                    for hl in range(HL):
                        ps = psum.tile([128, 2 * SC], F32, tag="mm",
                                       name=f"psqk_{b}_{tcn0}_{hl}_{id(dst)}")
                        if lead and dst is qT:
                            for ci in chunks:
                                for d in range(16):
                                    nc.tensor.matmul(
                                        ps[:, ci * SC : (ci + 1) * SC],
                                        w_t[:, hl, d], xts[ci][:, d],
                                        start=(d == 0), stop=(d == 15),
                                    )
                        else:
                            for d in range(16):
                                for ci in chunks:
                                    nc.tensor.matmul(
                                        ps[:, ci * SC : (ci + 1) * SC],
                                        w_t[:, hl, d], xts[ci][:, d],
                                        start=(d == 0), stop=(d == 15),
                                    )
                        for ci in chunks:
                            tcn = tcn0 + ci
                            nc.vector.tensor_scalar_add(
                                dst[:, hl, tcn * SC : (tcn + 1) * SC],
                                ps[:, ci * SC : (ci + 1) * SC], b_t[:, hl],
                            )
                for ci in chunks:
                    qkv_v_chunk(b, tcn0 + ci, xts[ci])

            def qkv_phase(b, pair_first):
                for p in range(2):
                    tcn0 = 2 * p
                    xts = (load_chunk(b, tcn0), load_chunk(b, tcn0 + 1))
                    if p == 0 and not pair_first:
                        # chunk-major: compute on chunk 0 covers chunk 1's DMA
                        qkv_single(b, tcn0, xts[0])
                        qkv_single(b, tcn0 + 1, xts[1])
                    else:
                        qkv_pair(b, tcn0, xts,
                                 lead=(not pair_first and p == 1))

            def attn_normalize(u, scn, o2, hl):
                for pj in range(2):  # s-tile pairs; one contiguous store
                    at2 = small.tile([128, 2 * 128], BF16, tag="at2",
                                     name=f"at2_{u}_{scn}_{pj}")
                    for s2 in range(2):
                        ss = 2 * pj + s2
                        o2t = o2[pj]
                        rc = small.tile([128, 1], F32, tag="rc",
                                        name=f"rc_{u}_{scn}_{ss}")
                        nc.vector.reciprocal(rc[:], o2t[:, s2, 128:129])
                        an = small.tile([128, 128], BF16, tag="an",
                                        name=f"an_{u}_{scn}_{ss}")
                        nc.vector.tensor_scalar_mul(
                            an[:], o2t[:, s2, 0:128], rc[:]
                        )
                        tp = psum.tile([128, 128], BF16, tag="tp", bufs=2,
                                       name=f"tp_{u}_{scn}_{ss}")
                        nc.tensor.transpose(tp[:], an[:], id_t[:])
                        nc.vector.tensor_scalar_add(
                            at2[:, s2 * 128 : (s2 + 1) * 128],
                            tp[:], bv_t[:, hl],
                        )
                    # gpsimd is safe: the collective doorbell is a brief
                    # WRITE, it does not block the gpsimd queue
                    nc.gpsimd.dma_start(a2a_in[u][2 * scn + pj], at2[:])

            def attention_unit(u):
                b, hl = u // HL, u % HL
                pending = None  # (scn, o2) awaiting normalize; emitted after
                # the NEXT scn's first scores+exp so the scalar engine has
                # work while the PE runs the normalize transposes.
                for scn in range(S // SC):
                    o2 = None
                    npair = 2 * scn + 2
                    for pr in range(npair):
                        sp = psum.tile([128, 2 * SC], F32, tag="mm",
                                       name=f"sp_{u}_{scn}_{pr}")
                        lives = []
                        for half in range(2):
                            tt = 2 * pr + half
                            off = max(0, tt - 4 * scn)
                            nlive = 4 - off
                            lives.append((tt, off, nlive))
                            s0 = scn * SC + off * 128
                            nc.tensor.matmul(
                                sp[:, half * SC : half * SC + nlive * 128],
                                kT[:, hl, tt * 128 : (tt + 1) * 128],
                                qT[:, hl, s0 : (scn + 1) * SC],
                                start=True, stop=True,
                            )
                        ex = expp.tile([128, 2 * SC], BF16, tag="ex",
                                       name=f"ex_{u}_{scn}_{pr}")
                        ncols = SC + lives[1][2] * 128  # through 2nd half live
                        nc.scalar.activation(
                            ex[:, :ncols], sp[:, :ncols],
                            mybir.ActivationFunctionType.Exp,
                            scale=INV_SQRT_DH,
                        )
                        for half in range(2):
                            tt, off, nlive = lives[half]
                            base = half * SC
                            if tt >= 4 * scn:  # diagonal: causal mask
                                nc.vector.tensor_mul(
                                    ex[:, base : base + 128],
                                    ex[:, base : base + 128], mask_t[:],
                                )
                        if pr == 0:
                            if pending is not None:
                                attn_normalize(u, pending[0], pending[1], hl)
                            o2 = [
                                psum.tile([128, 2, 129], F32, tag="o2",
                                          bufs=2, name=f"o2_{u}_{scn}_{i}")
                                for i in range(2)
                            ]
                        for half in range(2):
                            tt, off, nlive = lives[half]
                            base = half * SC
                            for ss in range(off, 4):
                                st = 4 * scn + ss
                                # start=True clears has_written BANK-wide;
                                # only the first matmul touching each o2 bank
                                # may set it (sibling overwrites rely on the
                                # cleared has_written bits).
                                nc.tensor.matmul(
                                    o2[ss // 2][:, ss % 2, :],
                                    ex[:, base + (ss - off) * 128
                                       : base + (ss - off + 1) * 128],
                                    vv[:, hl, tt],
                                    start=(tt == 0 and ss % 2 == 0),
                                    stop=(tt == st),
                                )
                    pending = (scn, o2)
                attn_normalize(u, pending[0], pending[1], hl)

            def a2a_fire(u):
                nc.gpsimd.collective_compute(
                    "AllToAll",
                    mybir.AluOpType.bypass,
                    ins=[a2a_in[u].opt()],
                    outs=[a2a_out[u].opt()],
                    replica_groups=[list(range(NC))],
                )

            def load_rcv(b, hl, rcv):
                # gpsimd: idle after its collective doorbells, and keeping
                # these off sync/scalar stops them blocking the out-stores
                for srcc in range(NC):
                    nc.gpsimd.dma_start(
                        rcv[:, srcc * HL + hl], a2a_out[b * HL + hl][srcc]
                    )

            def store_out(b, oc, ot):
                eng = nc.sync if oc % 2 == 0 else nc.scalar
                eng.dma_start(out[b, oc * 128 : (oc + 1) * 128, :], ot[:])

            def proj_phase(b, rcv):
                for oc in range(16):
                    ps = psum.tile([128, 2 * SC], F32, tag="mm",
                                   name=f"pso_{b}_{oc}")
                    for dt in range(16):
                        nc.tensor.matmul(
                            ps[:, :TOKB],
                            wo_t[:, dt, oc * 128 : (oc + 1) * 128],
                            rcv[:, dt],
                            start=(dt == 0), stop=(dt == 15),
                        )
                    ot = outp.tile([128, TOKB], BF16, tag="ot",
                                   name=f"ot_{b}_{oc}")
                    nc.vector.tensor_copy(ot[:], ps[:, :TOKB])
                    store_out(b, oc, ot)

            def proj_half_a(b, rcv, pp):
                # h0 (even dt) halves of every oc group -> bf16 partials;
                # runs while the last collective is in flight.
                for oc in range(16):
                    ps = psum.tile([128, 2 * SC], F32, tag="mm",
                                   name=f"psa_{b}_{oc}")
                    for i, dt in enumerate(range(0, 16, 2)):
                        nc.tensor.matmul(
                            ps[:, :TOKB],
                            wo_t[:, dt, oc * 128 : (oc + 1) * 128],
                            rcv[:, dt],
                            start=(i == 0), stop=(i == 7),
                        )
                    nc.vector.tensor_copy(pp[:, oc], ps[:, :TOKB])

            def proj_half_b(b, rcv, pp):
                for oc in range(16):
                    ps = psum.tile([128, 2 * SC], F32, tag="mm",
                                   name=f"psb_{b}_{oc}")
                    for i, dt in enumerate(range(1, 16, 2)):
                        nc.tensor.matmul(
                            ps[:, :TOKB],
                            wo_t[:, dt, oc * 128 : (oc + 1) * 128],
                            rcv[:, dt],
                            start=(i == 0), stop=(i == 7),
                        )
                    ot = outp.tile([128, TOKB], BF16, tag="ot",
                                   name=f"otb_{b}_{oc}")
                    nc.vector.tensor_add(ot[:], ps[:, :TOKB], pp[:, oc])
                    store_out(b, oc, ot)

            # ---------------- program order ----------------
            qkv_phase(0, pair_first=False)
            attention_unit(0)
            a2a_fire(0)
            attention_unit(1)
            a2a_fire(1)
            # W_out load: big (8.4 MB). The scheduler hoists dependency-free
            # DMAs to t=0 where this starved the startup-critical loads, so
            # give it a WAW dependency on late-qkv(0) data via a dummy write.
            wo_t = wopool.tile([128, 16, D], BF16, tag="wo")
            nc.vector.tensor_copy(wo_t[:, 0, 0:1], qT[:, 0, S - 1 : S])
            nc.scalar.dma_start(wo_t[:], w_out[:])
            # rcv0 loads fire as soon as each b0 per-head collective lands
            # (during b1 QKV/attention), so proj(b0) fills the tail bubble.
            rcv0 = rcvpool.tile([128, 16, TOKB], BF16, tag="rcv0")
            load_rcv(0, 0, rcv0)
            load_rcv(0, 1, rcv0)
            qkv_phase(1, pair_first=True)
            attention_unit(2)
            a2a_fire(2)
            attention_unit(3)
            a2a_fire(3)
            # emitted after the last doorbell: gpsimd is idle from here, so
            # these never queue ahead of attention's at2 stores
            rcv1 = rcvpool.tile([128, 16, TOKB], BF16, tag="rcv1")
            load_rcv(1, 0, rcv1)
            proj_phase(0, rcv0)
            load_rcv(1, 1, rcv1)
            pp1 = rcvpool.tile([128, 16, TOKB], BF16, tag="pp1")
            proj_half_a(1, rcv1, pp1)
            proj_half_b(1, rcv1, pp1)

    nc.compile()
    return nc


def make_in_maps(x, W_in, b_in, W_out, b_out):
    # xH[b, p, d, s] = x[b, s, d*128+p]  -> contiguous [128, 16, SC] chunk DMAs
    xH = np.ascontiguousarray(
        x.reshape(B, S, 16, 128).transpose(0, 3, 2, 1)
    ).astype(NPBF16)
    ident = np.eye(128, dtype=NPBF16)
    maskp = np.triu(np.ones((128, 128), dtype=np.float32)).astype(NPBF16)
    # w_out[p, dt, oc] = W_out[dt*128+p, oc] -> contiguous per-partition 64KB
    w_out_t = np.ascontiguousarray(
        W_out.reshape(16, 128, D).transpose(1, 0, 2)
    ).astype(NPBF16)

    in_maps = []
    for c in range(NC):
        hs = [2 * c + hl for hl in range(HL)]  # global head ids
        # wq[p, hl, d, m] = W_in[d*128+p, h*128+m]
        wq_c = np.ascontiguousarray(
            np.stack(
                [W_in[:, h * 128 : (h + 1) * 128].reshape(16, 128, 128) for h in hs]
            ).transpose(2, 0, 1, 3)
        ).astype(NPBF16)
        wk_c = np.ascontiguousarray(
            np.stack(
                [W_in[:, D + h * 128 : D + (h + 1) * 128].reshape(16, 128, 128)
                 for h in hs]
            ).transpose(2, 0, 1, 3)
        ).astype(NPBF16)
        wv_c = np.ascontiguousarray(
            np.concatenate(
                [
                    W_in[:, 2 * D + h * 128 : 2 * D + (h + 1) * 128].reshape(
                        16, 128, 128
                    )
                    for h in hs
                ],
                axis=2,
            ).transpose(1, 0, 2)
        ).astype(NPBF16)
        bq_c = np.stack([b_in[h * 128 : (h + 1) * 128] for h in hs], axis=1).reshape(
            128, HL, 1
        ).astype(np.float32)
        bk_c = np.stack(
            [b_in[D + h * 128 : D + (h + 1) * 128] for h in hs], axis=1
        ).reshape(128, HL, 1).astype(np.float32)
        bv_c = np.stack(
            [b_in[2 * D + h * 128 : 2 * D + (h + 1) * 128] for h in hs], axis=1
        ).reshape(128, HL, 1).astype(np.float32)
        in_maps.append(
            {
                "xH": xH,
                "wq": wq_c,
                "wk": wk_c,
                "wv": wv_c,
                "bq": bq_c,
                "bk": bk_c,
                "bv": bv_c,
                "w_out": w_out_t,
                "ident": ident,
                "maskp": maskp,
            }
        )
    return in_maps


_NC_CACHE = {}


def _get_nc(debug=False):
    key = f"nc{debug}"
    if key not in _NC_CACHE:
        _NC_CACHE[key] = build_kernel(debug)
    return _NC_CACHE[key]


def kernel(x, W_in, b_in, W_out, b_out, _trace=False, _debug=False, **kw):
    x = np.asarray(x, dtype=np.float32)
    W_in = np.asarray(W_in, dtype=np.float32)
    b_in = np.asarray(b_in, dtype=np.float32)
    W_out = np.asarray(W_out, dtype=np.float32)
    b_out = np.asarray(b_out, dtype=np.float32)

    nc = _get_nc(_debug)
    in_maps = make_in_maps(x, W_in, b_in, W_out, b_out)
    res = run_bass_kernel_spmd(nc, in_maps, core_ids=list(range(NC)), trace=_trace)
    outf = np.empty((B, S, D), dtype=np.float32)
    for c in range(NC):
        o = np.asarray(res.results[c]["out"]).astype(np.float32)  # [B, D, TOKB]
        for b in range(B):
            outf[b, c * TOKB : (c + 1) * TOKB, :] = o[b].T
    outf += b_out[None, None, :]
    if _trace or _debug:
        return outf, res
    return outf
